# revision 2
# baseline (speedup 1.0000x reference)
"""Self-contained 8-core Trainium2 Bass kernel for a 2-layer GATv2 encoder (v2).

Sharding: nodes partitioned across 8 NeuronCores by destination range; edges
dst-sorted and grouped into windows of 128 dst nodes (CSR-style, host side).

Key structure (all bf16 on the edge path, f32 PSUM accumulation):
  * Per layer, source/target transforms are computed ONCE into a combined
    DRAM table: rows [0,R) = xl = x @ Wl.T for every node, rows [R,R+XR) =
    xr = x_local @ Wr.T for local dst nodes. xl[src] rows are fetched by
    per-sub-block indirect DMA (128 rows each, the HW DGE limit); xr[dst]
    is broadcast per window by an indicator matmul (host-shipped one-hot).
  * z = e@We.T + xr[dst] + xl[src] is accumulated in PSUM by three matmuls
    per sub-block; the segment softmax denominator rides as 4 extra
    columns through the indicator (scatter) matmul. Messages are
    xl[src] * exp(logit) via one DVE op with a stride-0 broadcast AP.
  * Sub-blocks are processed in PAIRS sharing one PSUM bank so the
    activation/reduce/multiply work runs on [128,512] tiles (halves the
    per-instruction fixed costs).
  * Layer-1 -> layer-2 halo exchange: AllGather of transposed bf16 h,
    chunked in two so the first chunk overlaps the remaining edge windows.
  * DMA issue cost is spread: indirect gathers on gpsimd, all other loads/
    stores on the otherwise-idle sync (SP) queue.
  * `reps` identical iterations are compiled into the program so device
    time can be measured with the ~70ms axon dispatch latency amortized;
    dispatches are also issued async-pipelined.
"""
import sys

sys.path.insert(0, "/opt/trn_rl_repo")

import numpy as np

import concourse.bass as bass
import concourse.mybir as mybir
import concourse.tile as tile
from concourse.bass import IndirectOffsetOnAxis
from concourse.bass_utils import run_bass_kernel_spmd
from concourse.masks import make_identity

F32 = mybir.dt.float32
BF16 = mybir.dt.bfloat16
I32 = mybir.dt.int32
NPBF16 = mybir.dt.np(mybir.dt.bfloat16)

NCORES = 8
D_WIN = 128
HEADS = 4
NEG_SLOPE = 0.2
REPS_DEFAULT = 8
WIN_A = 25  # layer-1 windows in the first AllGather chunk


def _apply_tile_patch():
    """Pinned walrus rejects >2 sync waits on one CTRL instruction; split the
    TileContext exit drain's waits across a chain of drains."""
    from concourse.tile import ScopedClock

    if getattr(tile.TileContext, "_drain_patch_applied", False):
        return

    def _patched(self, tick_clock, wait_clock):
        nc = self.nc
        drain_inst = nc.sync.drain()
        wait_clock.add_sem_waits(
            drain_inst.ins, ScopedClock({None: tick_clock.global_clock})
        )
        ins = drain_inst.ins
        waits = list(ins.sync_info.on_wait)
        if len(waits) > 1:
            si = ins.sync_info
            si.on_wait = waits[:1]
            ins.sync_info = si
            for i in range(1, len(waits)):
                extra = nc.sync.drain()
                esi = extra.ins.sync_info
                if esi is None:
                    esi = mybir.SyncInfo(on_wait=[], on_update=[])
                esi.on_wait = waits[i : i + 1]
                extra.ins.sync_info = esi
        nc.all_engine_barrier()
        assert self.sems is not None
        popped = nc._tile_sem_poison_stack.pop()
        assert popped is self._sem_poison
        nc.clear_and_free_semaphores(list(self.sems.allocated().values()))
        nc.all_engine_barrier()

    tile.TileContext._drain_and_barrier = _patched
    tile.TileContext._drain_patch_applied = True


def _split_multi_waits(nc):
    """Pinned walrus accepts a single sync wait per instruction; move extra
    waits onto same-engine NoOps inserted immediately before."""
    cnt = 0
    for fn in nc.m.functions:
        for bb in fn.blocks:
            rebuilt = []
            changed = False
            for ins in bb.instructions:
                si = ins.sync_info
                if si is not None and si.on_wait is not None and len(si.on_wait) > 1:
                    waits = list(si.on_wait)
                    for w in waits[:-1]:
                        nop = mybir.InstNoOp(
                            name=f"WSPLIT-{cnt}", engine=ins.engine
                        )
                        cnt += 1
                        nop.sync_info = mybir.SyncInfo(on_wait=[w], on_update=[])
                        rebuilt.append(nop)
                    si.on_wait = [waits[-1]]
                    ins.sync_info = si
                    changed = True
                rebuilt.append(ins)
            if changed:
                bb.instructions[:] = rebuilt


def _preprocess(x, edge_index, edge_attr, n_loc):
    """Sort edges by dst, partition by dst range across cores, group into
    windows of 128 dst nodes, pad each window's edge list to a common cap.

    meta layouts are [128, n_win*2*nsub]: window k holds nsub columns of
    xl-row ids (src) then nsub columns of xr-row ids (table base + k*128 +
    drel). drel is [128, n_win*nsub] (dst - window start, -1 for padding).
    """
    n = x.shape[0]
    n_win = (n_loc + D_WIN - 1) // D_WIN
    xr_rows = n_win * D_WIN

    NB1 = (n + 127) // 128
    R1 = NB1 * 128
    R2 = NCORES * xr_rows

    src = np.asarray(edge_index[0]).astype(np.int64)
    dst = np.asarray(edge_index[1]).astype(np.int64)
    ea = np.asarray(edge_attr, dtype=np.float32)

    order = np.argsort(dst, kind="stable")
    src_s, dst_s, ea_s = src[order], dst[order], ea[order]
    core_of = dst_s // n_loc
    locdst = dst_s - core_of * n_loc
    win_of = locdst // D_WIN

    cnt = np.zeros((NCORES, n_win), dtype=np.int64)
    for c in range(NCORES):
        m = core_of == c
        cnt[c] = np.bincount(win_of[m], minlength=n_win)
    edge_cap = int(np.ceil(max(cnt.max(), 128) / 128) * 128)
    nsub = edge_cap // 128

    src_owner = src_s // n_loc
    src_row2 = src_owner * xr_rows + (src_s - src_owner * n_loc)

    per_core = []
    for c in range(NCORES):
        meta1 = np.zeros((128, n_win * nsub), dtype=np.int32)
        meta2 = np.zeros((128, n_win * nsub), dtype=np.int32)
        eat = np.zeros((n_win, 3, edge_cap), dtype=np.float32)
        m = core_of == c
        s1_c, s2_c, ld_c, w_c, ea_c = (
            src_s[m], src_row2[m], locdst[m], win_of[m], ea_s[m],
        )
        ssubh = np.zeros((n_win, 128, nsub * 128), dtype=NPBF16)
        ssubhT = np.zeros((n_win, 128, nsub * 128), dtype=NPBF16)
        for k in range(n_win):
            mk = w_c == k
            cnt_k = int(mk.sum())
            b1 = np.zeros(edge_cap, dtype=np.int64)
            b1[:cnt_k] = s1_c[mk]
            b2 = np.zeros(edge_cap, dtype=np.int64)
            b2[:cnt_k] = s2_c[mk]
            dr = np.full(edge_cap, -1.0, dtype=np.float32)
            dr[:cnt_k] = (ld_c[mk] - k * D_WIN).astype(np.float32)
            c0 = k * nsub
            meta1[:, c0 : c0 + nsub] = b1.reshape(nsub, 128).T
            meta2[:, c0 : c0 + nsub] = b2.reshape(nsub, 128).T
            drw = dr.reshape(nsub, 128).T  # [128 edges-in-part, nsub]
            onehot = (
                drw[:, :, None] == np.arange(128, dtype=np.float32)[None, None, :]
            ).astype(NPBF16)  # [128e, nsub, 128d]
            ssubh[k] = onehot.reshape(128, nsub * 128)
            ssubhT[k] = onehot.transpose(2, 1, 0).reshape(128, nsub * 128)
            eat[k, :, :cnt_k] = ea_c[mk].T
        per_core.append((meta1, meta2, ssubh, ssubhT, eat.astype(NPBF16)))
    return per_core, n_win, edge_cap, nsub, xr_rows


def _build_program(n, n_loc, n_win, edge_cap, nsub, xr_rows, reps=1):
    _apply_tile_patch()
    nc = bass.Bass()

    NB1 = (n + 127) // 128          # 391 blocks of global nodes (layer-1 tab)
    R1 = NB1 * 128                  # 50048
    NB2 = NCORES * n_win            # 392 blocks (layer-2 tab, padded rows)
    R2 = NB2 * 128                  # 50176
    XR = xr_rows                    # 6272
    NWS = n_win * nsub
    CA = WIN_A * 128                # first collective chunk columns
    CB = XR - CA

    xt = nc.dram_tensor("xt", [128, R1], BF16, kind="ExternalInput")
    xlocT = nc.dram_tensor("xlocT", [128, XR], BF16, kind="ExternalInput")
    meta1 = nc.dram_tensor("meta1", [128, NWS], I32, kind="ExternalInput")
    meta2 = nc.dram_tensor("meta2", [128, NWS], I32, kind="ExternalInput")
    ssubh = nc.dram_tensor("ssubh", [n_win, 128, nsub * 128], BF16, kind="ExternalInput")
    ssubhT = nc.dram_tensor("ssubhT", [n_win, 128, nsub * 128], BF16, kind="ExternalInput")
    eat = nc.dram_tensor("eat", [n_win, 3, edge_cap], BF16, kind="ExternalInput")

    wlt1 = nc.dram_tensor("wlt1", [128, 256], BF16, kind="ExternalInput")
    wrt1 = nc.dram_tensor("wrt1", [128, 256], BF16, kind="ExternalInput")
    wet1 = nc.dram_tensor("wet1", [3, 256], BF16, kind="ExternalInput")
    attrep1 = nc.dram_tensor("attrep1", [128, 256], BF16, kind="ExternalInput")
    b1rep = nc.dram_tensor("b1rep", [128, 64], F32, kind="ExternalInput")
    wlt2 = nc.dram_tensor("wlt2", [64, 256], BF16, kind="ExternalInput")
    wrt2 = nc.dram_tensor("wrt2", [64, 256], BF16, kind="ExternalInput")
    wet2 = nc.dram_tensor("wet2", [3, 256], BF16, kind="ExternalInput")
    attrep2 = nc.dram_tensor("attrep2", [128, 256], BF16, kind="ExternalInput")
    b2rep = nc.dram_tensor("b2rep", [128, 64], F32, kind="ExternalInput")
    prw = nc.dram_tensor("prw", [128, 64], F32, kind="ExternalInput")

    out_loc = nc.dram_tensor("out_loc", [XR, 64], F32, kind="ExternalOutput")

    # sub-block spans: pairs (width 2) plus a tail single if nsub is odd
    spans = []
    s0 = 0
    while s0 < nsub:
        w = 2 if s0 + 2 <= nsub else 1
        spans.append((s0, w))
        s0 += w

    with tile.TileContext(nc) as tc:
        from contextlib import ExitStack

        with ExitStack() as ctx:
            const = ctx.enter_context(tc.tile_pool(name="const", bufs=1))
            dram = ctx.enter_context(tc.tile_pool(name="dram", bufs=1, space="DRAM"))
            hfull_p = ctx.enter_context(
                tc.tile_pool(name="hfull", bufs=min(reps, 2), space="DRAM")
            )
            work = ctx.enter_context(tc.tile_pool(name="work", bufs=3))
            meta_p = ctx.enter_context(tc.tile_pool(name="meta", bufs=2))
            psum_z = ctx.enter_context(tc.tile_pool(name="psz", bufs=2, space="PSUM"))
            psum_t = ctx.enter_context(tc.tile_pool(name="pst", bufs=2, space="PSUM"))
            pacc_p = ctx.enter_context(tc.tile_pool(name="pacc", bufs=2, space="PSUM"))

            ident_bf = const.tile([128, 128], BF16, tag="ident_bf")
            make_identity(nc, ident_bf[:])

            def load_const(t, shape, dt):
                s = const.tile(shape, dt, tag=t.name)
                nc.sync.dma_start(out=s[:], in_=t[:])
                return s

            wlt1_s = load_const(wlt1, [128, 256], BF16)
            wrt1_s = load_const(wrt1, [128, 256], BF16)
            wet1_s = load_const(wet1, [3, 256], BF16)
            attrep1_s = load_const(attrep1, [128, 256], BF16)
            b1rep_s = load_const(b1rep, [128, 64], F32)
            wlt2_s = load_const(wlt2, [64, 256], BF16)
            wrt2_s = load_const(wrt2, [64, 256], BF16)
            wet2_s = load_const(wet2, [3, 256], BF16)
            attrep2_s = load_const(attrep2, [128, 256], BF16)
            b2rep_s = load_const(b2rep, [128, 64], F32)
            prw_s = load_const(prw, [128, 64], F32)

            # Persistent SBUF tiles (rewritten every rep)
            hT_sb = const.tile([64, XR], BF16, tag="hT_sb")
            meta1_sb = const.tile([128, NWS], I32, tag="meta1_sb")
            meta2_sb = const.tile([128, NWS], I32, tag="meta2_sb")
            xloc_sb = const.tile([128, XR], BF16, tag="xloc_sb")

            ctab1 = dram.tile([R1 + XR, 256], BF16, tag="ctab1")
            ctab2 = dram.tile([R2 + XR, 256], BF16, tag="ctab2")
            hT_locA = dram.tile([64, CA], BF16, tag="hT_locA")
            hT_locB = dram.tile([64, CB], BF16, tag="hT_locB")

            def copy_psum(j, out_ap, in_ap):
                """psum->sbuf copy, alternating Act/DVE (Pool cannot read PSUM)."""
                if j % 4 != 3:
                    nc.scalar.copy(out=out_ap, in_=in_ap)
                else:
                    nc.vector.tensor_copy(out=out_ap, in_=in_ap)

            def build_tab(groups, w_s, out_dram, row0=0):
                """Transform table build. groups: list of (src_ap, pdim, nblk).
                Table rows [row0 + j*128 ...] = chunk_j^T @ w_s, G blocks per
                load/store DMA. src_ap None means SBUF source given per-block
                by sbuf_src(j)."""
                j0 = 0
                for gi, (src_ap, pdim, nblk, sbuf_src) in enumerate(groups):
                    if src_ap is not None:
                        lt = work.tile([pdim, nblk * 128], BF16, tag="lt")
                        nc.sync.dma_start(out=lt[:], in_=src_ap)
                    xo = work.tile([128, nblk * 256], BF16, tag="xo")
                    for i in range(nblk):
                        if src_ap is not None:
                            lhs = lt[:, i * 128 : (i + 1) * 128]
                        else:
                            lhs = sbuf_src(j0 + i)
                        pb = psum_z.tile([128, 256], F32, tag="pb")
                        nc.tensor.matmul(
                            pb[:], lhsT=lhs, rhs=w_s[:], start=True, stop=True
                        )
                        copy_psum(j0 + i, xo[:, i * 256 : (i + 1) * 256], pb[:])
                    r0 = row0 + j0 * 128
                    nc.sync.dma_start(
                        out=out_dram[r0 : r0 + nblk * 128, :].rearrange(
                            "(g p) c -> p g c", g=nblk
                        ),
                        in_=xo[:].rearrange("p (g c) -> p g c", c=256),
                    )
                    j0 += nblk

            G = 8

            def grouped(total, src_of):
                """[(src_ap, pdim, nblk, None)] with nblk<=G from src_of(j0, nblk)."""
                out = []
                j0 = 0
                while j0 < total:
                    nblk = min(G, total - j0)
                    ap, pdim = src_of(j0, nblk)
                    out.append((ap, pdim, nblk, None))
                    j0 += nblk
                return out

            def edge_layer(ctab, xr_row0, meta_sb, wet_s, attrep_s, brep_s, final,
                           after_window=None):
                for k in range(n_win):
                    eat_t = meta_p.tile([3, edge_cap], BF16, tag="eat")
                    nc.sync.dma_start(out=eat_t[:], in_=eat[k])
                    xrw_t = meta_p.tile([128, 256], BF16, tag="xrw")
                    nc.sync.dma_start(
                        out=xrw_t[:],
                        in_=ctab[xr_row0 + k * 128 : xr_row0 + (k + 1) * 128, :],
                    )
                    xls = meta_p.tile([128, nsub * 256], BF16, tag="xls")
                    for s in range(nsub):
                        nc.gpsimd.indirect_dma_start(
                            out=xls[:, s * 256 : (s + 1) * 256],
                            out_offset=None,
                            in_=ctab[:],
                            in_offset=IndirectOffsetOnAxis(
                                ap=meta_sb[:, k * nsub + s : k * nsub + s + 1], axis=0
                            ),
                        )
                    ssub_t = meta_p.tile([128, nsub * 128], BF16, tag="ssub_t")
                    nc.sync.dma_start(out=ssub_t[:], in_=ssubh[k])
                    ssubT_t = meta_p.tile([128, nsub * 128], BF16, tag="ssubT_t")
                    nc.sync.dma_start(out=ssubT_t[:], in_=ssubhT[k])
                    pacc = pacc_p.tile([128, 260], F32, tag="pacc")
                    for s0, w in spans:
                        pz = psum_z.tile([128, 512], F32, tag="pz")
                        for i in range(w):
                            s = s0 + i
                            o = pz[:, i * 256 : (i + 1) * 256]
                            nc.tensor.matmul(
                                o,
                                lhsT=eat_t[:, s * 128 : (s + 1) * 128],
                                rhs=wet_s[:],
                                start=True,
                                stop=False,
                                skip_group_check=True,
                            )
                            nc.tensor.matmul(
                                o,
                                lhsT=ssubT_t[:, s * 128 : (s + 1) * 128],
                                rhs=xrw_t[:],
                                start=False,
                                stop=False,
                                skip_group_check=True,
                            )
                            nc.tensor.matmul(
                                o,
                                lhsT=ident_bf[:],
                                rhs=xls[:, s * 256 : (s + 1) * 256],
                                start=False,
                                stop=True,
                                skip_group_check=True,
                            )
                        z2 = work.tile([128, w * 256], BF16, tag="z2")
                        nc.scalar.activation(
                            out=z2[:],
                            in_=pz[:, 0 : w * 256],
                            func=mybir.ActivationFunctionType.Prelu,
                            alpha=NEG_SLOPE,
                        )
                        zw2 = work.tile([128, w * 256], BF16, tag="zw2")
                        nc.vector.tensor_tensor(
                            out=zw2[:].rearrange("p (s c) -> p s c", c=256),
                            in0=z2[:].rearrange("p (s c) -> p s c", c=256),
                            in1=attrep_s[:, None, :].to_broadcast([128, w, 256]),
                            op=mybir.AluOpType.mult,
                        )
                        msgw = work.tile([128, w * 260], BF16, tag="msgw")
                        mview = msgw[:].rearrange("p (s r) -> p s r", r=260)
                        with nc.allow_low_precision(reason="bf16 attn logits"):
                            nc.vector.reduce_sum(
                                out=mview[:, :, 256:260],
                                in_=zw2[:].rearrange(
                                    "p (s h c) -> p s h c", h=4, c=64
                                ),
                                axis=mybir.AxisListType.X,
                            )
                        nc.scalar.activation(
                            out=mview[:, :, 256:260],
                            in_=mview[:, :, 256:260],
                            func=mybir.ActivationFunctionType.Exp,
                        )
                        nc.vector.tensor_tensor(
                            out=mview[:, :, 0:256].rearrange(
                                "p s (h c) -> p s h c", c=64
                            ),
                            in0=xls[:, s0 * 256 : (s0 + w) * 256].rearrange(
                                "p (s h c) -> p s h c", h=4, c=64
                            ),
                            in1=mview[:, :, 256:260][:, :, :, None].to_broadcast(
                                [128, w, 4, 64]
                            ),
                            op=mybir.AluOpType.mult,
                        )
                        for i in range(w):
                            s = s0 + i
                            nc.tensor.matmul(
                                pacc[:],
                                lhsT=ssub_t[:, (s0 + i) * 128 : (s0 + i + 1) * 128],
                                rhs=msgw[:, i * 260 : (i + 1) * 260],
                                start=(s == 0),
                                stop=(s == nsub - 1),
                            )
                    # ---- per-window epilogue ----
                    den = work.tile([128, 4], F32, tag="den")
                    nc.vector.tensor_scalar(
                        out=den[:],
                        in0=pacc[:, 256:260],
                        scalar1=float(HEADS),
                        scalar2=4e-16,
                        op0=mybir.AluOpType.mult,
                        op1=mybir.AluOpType.add,
                    )
                    rec = work.tile([128, 4], F32, tag="rec")
                    nc.vector.reciprocal(out=rec[:], in_=den[:])
                    hm = work.tile([128, 256], F32, tag="hm")
                    nc.vector.tensor_tensor(
                        out=hm[:].rearrange("p (h c) -> p h c", c=64),
                        in0=pacc[:, 0:256].rearrange("p (h c) -> p h c", c=64),
                        in1=rec[:, :, None].to_broadcast([128, 4, 64]),
                        op=mybir.AluOpType.mult,
                    )
                    hsum = work.tile([128, 64], F32, tag="hsum")
                    nc.vector.reduce_sum(
                        out=hsum[:],
                        in_=hm[:].rearrange("p (h c) -> p c h", c=64),
                        axis=mybir.AxisListType.X,
                    )
                    if final:
                        ht = work.tile([128, 64], F32, tag="ht")
                        nc.vector.tensor_tensor(
                            out=ht[:], in0=hsum[:], in1=brep_s[:], op=mybir.AluOpType.add
                        )
                        pos = work.tile([128, 64], F32, tag="pos")
                        nc.vector.tensor_scalar(
                            out=pos[:], in0=ht[:], scalar1=0.0, scalar2=None,
                            op0=mybir.AluOpType.max,
                        )
                        neg = work.tile([128, 64], F32, tag="neg")
                        nc.vector.tensor_scalar(
                            out=neg[:], in0=ht[:], scalar1=0.0, scalar2=None,
                            op0=mybir.AluOpType.min,
                        )
                        negw = work.tile([128, 64], F32, tag="negw")
                        nc.vector.tensor_tensor(
                            out=negw[:], in0=neg[:], in1=prw_s[:], op=mybir.AluOpType.mult
                        )
                        fin = work.tile([128, 64], F32, tag="fin")
                        nc.vector.tensor_tensor(
                            out=fin[:], in0=pos[:], in1=negw[:], op=mybir.AluOpType.add
                        )
                        nc.sync.dma_start(
                            out=out_loc[k * 128 : (k + 1) * 128, :], in_=fin[:]
                        )
                    else:
                        htb = work.tile([128, 64], BF16, tag="htb")
                        nc.vector.tensor_tensor(
                            out=htb[:], in0=hsum[:], in1=brep_s[:], op=mybir.AluOpType.add
                        )
                        pT2 = psum_t.tile([64, 128], BF16, tag="pT2")
                        nc.tensor.transpose(out=pT2[:], in_=htb[:], identity=ident_bf[:])
                        nc.vector.tensor_copy(
                            out=hT_sb[:, k * 128 : (k + 1) * 128], in_=pT2[:]
                        )
                    if after_window is not None:
                        after_window(k)

            for _rep in range(reps):
                hT_fullA = hfull_p.tile(
                    [NCORES * 64, CA], BF16, addr_space="Shared", tag="hT_fullA"
                )
                hT_fullB = hfull_p.tile(
                    [NCORES * 64, CB], BF16, addr_space="Shared", tag="hT_fullB"
                )
                # ---- per-rep input staging (graph structure + local x) ----
                nc.sync.dma_start(out=meta1_sb[:], in_=meta1[:])
                nc.sync.dma_start(out=meta2_sb[:], in_=meta2[:])
                nc.sync.dma_start(out=xloc_sb[:], in_=xlocT[:])
                # ---- layer-1 tables (combined: xl rows then xr rows) ----
                build_tab(
                    grouped(NB1, lambda j0, nb: (
                        xt[:, j0 * 128 : (j0 + nb) * 128], 128)),
                    wlt1_s,
                    ctab1[:],
                    row0=0,
                )
                build_tab(
                    [(None, 128, nb, (lambda j: xloc_sb[:, j * 128 : (j + 1) * 128]))
                     for nb in [G] * (n_win // G) + ([n_win % G] if n_win % G else [])],
                    wrt1_s,
                    ctab1[:],
                    row0=R1,
                )

                # ---- layer-1 edges, with chunked halo exchange ----
                def after_win(k):
                    if k == WIN_A - 1:
                        nc.sync.dma_start(out=hT_locA[:], in_=hT_sb[:, 0:CA])
                        nc.gpsimd.collective_compute(
                            "AllGather",
                            mybir.AluOpType.bypass,
                            replica_groups=[list(range(NCORES))],
                            ins=[hT_locA[:]],
                            outs=[hT_fullA[:]],
                        )
                    elif k == n_win - 1:
                        nc.sync.dma_start(out=hT_locB[:], in_=hT_sb[:, CA:XR])

                edge_layer(
                    ctab1, R1, meta1_sb, wet1_s, attrep1_s, b1rep_s, False,
                    after_window=after_win,
                )
                # xr2 only needs local hT: build before the second collective
                # chunk so it overlaps the transfer.
                build_tab(
                    [(None, 64, nb, (lambda j: hT_sb[:, j * 128 : (j + 1) * 128]))
                     for nb in [G] * (n_win // G) + ([n_win % G] if n_win % G else [])],
                    wrt2_s,
                    ctab2[:],
                    row0=R2,
                )
                nc.gpsimd.collective_compute(
                    "AllGather",
                    mybir.AluOpType.bypass,
                    replica_groups=[list(range(NCORES))],
                    ins=[hT_locB[:]],
                    outs=[hT_fullB[:]],
                )
                # ---- layer-2 xl table: blocks (c, k) from gathered hT ----
                l2_groups = []
                for c in range(NCORES):
                    k0 = 0
                    while k0 < n_win:
                        if k0 < WIN_A:
                            nb = min(G, WIN_A - k0)
                            ap = hT_fullA[
                                c * 64 : (c + 1) * 64, k0 * 128 : (k0 + nb) * 128
                            ]
                        else:
                            nb = min(G, n_win - k0)
                            ap = hT_fullB[
                                c * 64 : (c + 1) * 64,
                                (k0 - WIN_A) * 128 : (k0 - WIN_A + nb) * 128,
                            ]
                        l2_groups.append((ap, 64, nb, None))
                        k0 += nb
                build_tab(l2_groups, wlt2_s, ctab2[:], row0=0)
                # ---- layer-2 edges ----
                edge_layer(ctab2, R2, meta2_sb, wet2_s, attrep2_s, b2rep_s, True)

    _split_multi_waits(nc)
    return nc


_CACHE = {}


def _get_program(key, *args, **kwargs):
    if key not in _CACHE:
        _CACHE[key] = _build_program(*args, **kwargs)
    return _CACHE[key]


def _prep_inputs(x, edge_index, edge_attr, Wl1, Wr1, We1, att1, b1, Wl2, Wr2,
                 We2, att2, b2, prelu_w):
    x = np.ascontiguousarray(np.asarray(x, dtype=np.float32))
    n = x.shape[0]
    assert n % NCORES == 0
    n_loc = n // NCORES

    per_core, n_win, edge_cap, nsub, xr_rows = _preprocess(
        x, edge_index, edge_attr, n_loc
    )

    NB1 = (n + 127) // 128
    R1 = NB1 * 128
    xt_h = np.zeros((128, R1), dtype=NPBF16)
    xt_h[:, :n] = x.T.astype(NPBF16)

    def prep_w(W):
        return np.ascontiguousarray(np.asarray(W, dtype=np.float32).T).astype(NPBF16)

    wlt1_h, wrt1_h, wet1_h = prep_w(Wl1), prep_w(Wr1), prep_w(We1)
    wlt2_h, wrt2_h, wet2_h = prep_w(Wl2), prep_w(Wr2), prep_w(We2)
    attrep1_h = np.broadcast_to(
        np.asarray(att1, np.float32).reshape(1, -1), (128, 256)
    ).astype(NPBF16)
    attrep2_h = np.broadcast_to(
        np.asarray(att2, np.float32).reshape(1, -1), (128, 256)
    ).astype(NPBF16)
    b1rep_h = np.broadcast_to(np.asarray(b1, np.float32).reshape(1, -1), (128, 64)).copy()
    b2rep_h = np.broadcast_to(np.asarray(b2, np.float32).reshape(1, -1), (128, 64)).copy()
    prw_h = np.broadcast_to(
        np.asarray(prelu_w, np.float32).reshape(1, -1), (128, 64)
    ).copy()

    in_maps = []
    for c in range(NCORES):
        meta1, meta2, ssubh_h, ssubhT_h, eat = per_core[c]
        xloc = np.zeros((128, xr_rows), dtype=NPBF16)
        xloc[:, :n_loc] = x[c * n_loc : (c + 1) * n_loc].T.astype(NPBF16)
        in_maps.append(
            {
                "xt": xt_h,
                "xlocT": xloc,
                "meta1": meta1,
                "meta2": meta2,
                "ssubh": ssubh_h,
                "ssubhT": ssubhT_h,
                "eat": eat,
                "wlt1": wlt1_h,
                "wrt1": wrt1_h,
                "wet1": wet1_h,
                "attrep1": attrep1_h,
                "b1rep": b1rep_h,
                "wlt2": wlt2_h,
                "wrt2": wrt2_h,
                "wet2": wet2_h,
                "attrep2": attrep2_h,
                "b2rep": b2rep_h,
                "prw": prw_h,
            }
        )
    return in_maps, n, n_loc, n_win, edge_cap, nsub, xr_rows


def run_gnn(x, edge_index, edge_attr, Wl1, Wr1, We1, att1, b1, Wl2, Wr2, We2,
            att2, b2, prelu_w, reps=REPS_DEFAULT, trace=False):
    in_maps, n, n_loc, n_win, edge_cap, nsub, xr_rows = _prep_inputs(
        x, edge_index, edge_attr, Wl1, Wr1, We1, att1, b1, Wl2, Wr2, We2, att2,
        b2, prelu_w,
    )
    nc = _get_program(
        (n, n_loc, n_win, edge_cap, nsub, reps),
        n, n_loc, n_win, edge_cap, nsub, xr_rows, reps=reps,
    )
    global _last_in_maps
    _last_in_maps = in_maps
    res = run_bass_kernel_spmd(nc, in_maps, core_ids=list(range(NCORES)), trace=trace)
    out = np.empty((n, 64), dtype=np.float32)
    for c in range(NCORES):
        out[c * n_loc : (c + 1) * n_loc] = res.results[c]["out_loc"][:n_loc]
    if trace:
        return out, res
    return out


def timed_run(in_maps, nc, n_iters=3, async_k=1):
    """Device-resident inputs; time repeated executions. With async_k>1,
    issue async_k dispatches asynchronously and block once (amortizes the
    axon tunnel round-trip latency)."""
    import time as _time

    import jax
    from jax.sharding import Mesh, PartitionSpec, NamedSharding
    from jax.experimental.shard_map import shard_map

    from concourse import bass2jax as b2j
    from concourse import mybir as _mybir

    b2j.install_neuronx_cc_hook()
    partition_name = nc.partition_id_tensor.name if nc.partition_id_tensor else None
    in_names, out_names, out_avals = [], [], []
    for alloc in nc.m.functions[0].allocations:
        if not isinstance(alloc, _mybir.MemoryLocationSet):
            continue
        name = alloc.memorylocations[0].name
        if alloc.kind == "ExternalInput":
            if name != partition_name:
                in_names.append(name)
        elif alloc.kind == "ExternalOutput":
            out_names.append(name)
            out_avals.append(
                jax.core.ShapedArray(tuple(alloc.tensor_shape), _mybir.dt.np(alloc.dtype))
            )
    n_params = len(in_names)
    zero_outs = [np.zeros(a.shape, a.dtype) for a in out_avals]
    all_names = in_names + out_names + ([partition_name] if partition_name else [])

    def _body(*args):
        operands = list(args)
        if partition_name is not None:
            operands.append(b2j.partition_id_tensor())
        return tuple(
            b2j._bass_exec_p.bind(
                *operands,
                out_avals=tuple(out_avals),
                in_names=tuple(all_names),
                out_names=tuple(out_names),
                lowering_input_output_aliases=(),
                sim_require_finite=True,
                sim_require_nnan=True,
                nc=nc,
            )
        )

    devices = jax.devices()[:NCORES]
    mesh = Mesh(np.asarray(devices), ("core",))
    spec = PartitionSpec("core")
    n_out = len(out_names)
    sharded = jax.jit(
        shard_map(
            _body,
            mesh=mesh,
            in_specs=(spec,) * (n_params + n_out),
            out_specs=(spec,) * n_out,
            check_rep=False,
        ),
        keep_unused=True,
    )
    sh = NamedSharding(mesh, spec)
    dev_in = [
        jax.device_put(
            np.concatenate([np.asarray(in_maps[c][nm]) for c in range(NCORES)], axis=0), sh
        )
        for nm in in_names
    ]
    dev_zero = [
        jax.device_put(
            np.zeros((NCORES * z.shape[0], *z.shape[1:]), z.dtype), sh
        )
        for z in zero_outs
    ]
    outs = sharded(*dev_in, *dev_zero)
    jax.block_until_ready(outs)
    times = []
    for _ in range(n_iters):
        t0 = _time.perf_counter()
        pend = [sharded(*dev_in, *dev_zero) for _ in range(async_k)]
        jax.block_until_ready(pend)
        times.append((_time.perf_counter() - t0) / async_k)
        outs = pend[-1]
    out_np = [np.asarray(o) for o in outs]
    results = [
        {nm: out_np[i].reshape(NCORES, *out_avals[i].shape)[c] for i, nm in enumerate(out_names)}
        for c in range(NCORES)
    ]
    return results, times


def kernel(**inputs):
    return run_gnn(
        inputs["x"],
        inputs["edge_index"],
        inputs["edge_attr"],
        inputs["Wl1"],
        inputs["Wr1"],
        inputs["We1"],
        inputs["att1"],
        inputs["b1"],
        inputs["Wl2"],
        inputs["Wr2"],
        inputs["We2"],
        inputs["att2"],
        inputs["b2"],
        inputs["prelu_w"],
    )
